# revision 20
# baseline (speedup 1.0000x reference)
"""Trainium2 Bass kernel for the 3-level soft decision-tree head.

Math (see reference): with pen = x,
  u1 = x @ W_final + b_final                            (B, 8)
  l1 = x @ Wp1 + u1*wu1 + b1 ; p1 = softmax(l1)         (B, 8, 6)
  u2 = p1 * u1
  l2 = x @ Wp2 + u2*wu2 + b2 ; p2 = softmax(l2)         (B, 8, 6, 4)
  u3 = p2 * u2
  out = concat([u1, u2.flat, u3.flat])                  (B, 248)

Key restructurings:
  * The u1*wu1 term is linear in x, so it folds into the level-1 weights
    host-side: Wp1_eff[i,:,c] = Wp1[i,:,c] + W_final[:,i]*wu1[i,c] (and
    b1_eff = b1 + b_final*wu1). All three linears then stack into one
    (1280, 248) matrix -> a single x @ W_all matmul feeds every level.
  * The matmul runs in fp16 (1 cycle/row on the PE + fast weight load;
    fp32/f32r stream at <=half rate). x is loaded from HBM in fp32 and
    cast to fp16 on-chip (ScalarE/GpSimd, which are otherwise idle).
    fp16 keeps 11 mantissa bits; inputs are O(1) and weights O(0.02), so
    there is no range risk and the output error stays ~1e-4 relative.
  * Sharding: pure data parallel, batch 16384 -> 8 x 2048 (one shard per
    NeuronCore). Weights (<2MB) are replicated. The x shard is staged
    host-side in [D, Bc] (transposed) layout so the device can DMA
    [d-on-partition] matmul operand tiles directly; bytes moved are
    identical to the natural layout.
  * Nonzero biases would enter through one extra K=1 matmul (ones row x
    bias row); it is skipped when the biases are all zero (as here).
  * Softmax skips the max-subtraction: logits are O(1) by construction,
    exp cannot overflow, and softmax is shift-invariant so the result is
    unchanged up to fp32 rounding.
"""

import sys

sys.path.insert(0, "/opt/trn_rl_repo")

from contextlib import ExitStack

import numpy as np

import concourse.bass as bass  # noqa: F401
import concourse.mybir as mybir
import concourse.tile as tile
from concourse import bacc, bass_utils

B, D = 16384, 1280
C1, C2, C3 = 8, 6, 4
N1 = C1                 # 8    level-1 (root) outputs
N2 = C1 * C2            # 48   level-2 logits
N3 = C1 * C2 * C3       # 192  level-3 logits
F = N1 + N2 + N3        # 248  output columns
FP = 256                # matmul free dim padded to a friendly size
NCORES = 8
BC = B // NCORES        # 2048 batch rows per core
NGROUPS = 4
R = BC // NGROUPS       # 512 rows per pipeline group
S = R // 128            # 4 subtiles of 128 rows
KC = D // 128           # 10 contraction chunks

f32 = mybir.dt.float32
f16 = mybir.dt.float16

LAST_RESULT = None      # BassKernelResults of the most recent run (for test.py)
GROUPS = None           # set while building: (row0, nrows, pieces)
_CACHED_NC = {}


def _build_body(ctx, tc, nc, xt, w, wu2r, br, onesr, y, with_bias):
    const_pool = ctx.enter_context(tc.tile_pool(name="const", bufs=1))
    xf_pool = ctx.enter_context(tc.tile_pool(name="xf", bufs=1))
    xh_pool = ctx.enter_context(tc.tile_pool(name="xh", bufs=1))
    psum_pool = ctx.enter_context(tc.tile_pool(name="ps", bufs=2, space="PSUM"))
    warm_pool = ctx.enter_context(tc.tile_pool(name="warm", bufs=1, space="PSUM"))
    ep_pool = ctx.enter_context(tc.tile_pool(name="ep", bufs=2))
    out_pool = ctx.enter_context(tc.tile_pool(name="out", bufs=2))

    # Resident constants (issued on the ACT HWDGE ring so the SyncE ring is
    # free to start streaming x immediately)
    w_sb = const_pool.tile([128, KC * FP], f16)
    w_sb3 = w_sb[:].rearrange("p (c n) -> p c n", c=KC)
    nc.scalar.dma_start(w_sb3, w.rearrange("(c p) n -> p c n", p=128))
    wu2_sb = const_pool.tile([128, N3], f32)
    nc.scalar.dma_start(wu2_sb[:], wu2r)
    if with_bias:
        br_sb = const_pool.tile([1, FP], f16)
        nc.scalar.dma_start(br_sb[:], br)
        ones_sb = const_pool.tile([1, 128], f16)
        nc.scalar.dma_start(ones_sb[:], onesr)

    xt_v = xt.rearrange("(c p) b -> p c b", p=128)

    # ---- group schedule: (row0, nrows, load pieces as chunk ranges).
    # Group 0 starts with small pieces so the PE gets going early; the LAST
    # group is small with a tiny final piece, because everything loaded last
    # sits on the kernel's critical tail (cast -> matmul -> softmax -> store
    # all serialize after the final byte arrives).
    global GROUPS
    GROUPS = [
        (0,    640, [(0, 2), (2, 5), (5, 8), (8, 10)]),
        (640,  640, [(0, 5), (5, 10)]),
        (1280, 512, [(0, 5), (5, 10)]),
        (1792, 256, [(0, 4), (4, 8), (8, 10)]),
    ]

    # Everything is prefetched (whole shard fits in SBUF) so the PE never
    # starves once running. Casts alternate DVE/ACT; the last group's casts
    # stay on DVE (cheaper per op -> shorter tail).
    xh_views = {}           # g -> list of (chunk_lo, 3d fp16 view)
    cast_flip = 0
    for g, (r0, nr, pieces) in enumerate(GROUPS):
        views = []
        for (c0, c1) in pieces:
            nch = c1 - c0
            xf = xf_pool.tile([128, nch * nr], f32, tag=f"xf{g}_{c0}")
            xf3 = xf[:].rearrange("p (c b) -> p c b", c=nch)
            nc.sync.dma_start(xf3, xt_v[:, c0:c1, r0:r0 + nr])
            xh = xh_pool.tile([128, nch * nr], f16, tag=f"xh{g}_{c0}")
            if g == len(GROUPS) - 1 or cast_flip % 2 == 0:
                nc.vector.tensor_copy(xh[:], xf[:])
            else:
                nc.scalar.copy(xh[:], xf[:])
            cast_flip += 1
            views.append((c0, xh[:].rearrange("p (c b) -> p c b", c=nch)))
        xh_views[g] = views

    def lhsT(g, k, s):
        for c0, v in xh_views[g]:
            if c0 <= k < c0 + v.shape[1]:
                return v[:, k - c0, s * 128:(s + 1) * 128]
        raise AssertionError

    # ---- PE warm-up: ~45 throwaway matmuls on the (early-resident) weight
    # tile keep TensorE busy >3.4us while x streams in, so the HAM clock
    # gate opens (1.2 -> 2.4 GHz) before the first real matmul. PE is
    # otherwise idle during this window; micro-gaps later never reach the
    # ~3.4us re-throttle window, so the array stays warm for the whole run.
    warm_ps = warm_pool.tile([128, FP], f32)
    for _ in range(45):
        nc.tensor.matmul(warm_ps[:], w_sb3[:, 0, 0:128], w_sb3[:, 0, :],
                         start=True, stop=True)

    for g, (r0, nr, pieces) in enumerate(GROUPS):
        ns_g = nr // 128
        ps = psum_pool.tile([128, ns_g * FP], f32, tag="ps")
        ps3 = ps[:].rearrange("p (s n) -> p s n", s=ns_g)
        for s in range(ns_g):
            for k in range(KC):
                nc.tensor.matmul(
                    ps3[:, s, :],
                    lhsT(g, k, s),
                    w_sb3[:, k, :],
                    start=(k == 0),
                    stop=(not with_bias) and (k == KC - 1),
                )
            if with_bias:
                nc.tensor.matmul(
                    ps3[:, s, :],
                    ones_sb[0:1, :],
                    br_sb[0:1, :],
                    start=False,
                    stop=True,
                )

        # ---- epilogue + store (bank-pair halves for the bigger groups so
        # DVE work granularity stays moderate)
        if ns_g > 4:
            h = ns_g // 2
            _epilogue(nc, ep_pool, out_pool, wu2_sb, ps3, y, r0, 0, h)
            _epilogue(nc, ep_pool, out_pool, wu2_sb, ps3, y, r0, h, ns_g - h)
        else:
            _epilogue(nc, ep_pool, out_pool, wu2_sb, ps3, y, r0, 0, ns_g)


def _epilogue(nc, ep_pool, out_pool, wu2_sb, ps3, y, r_base, s0, ns):
    """Softmax levels 1+2 and output assembly for subtiles [s0, s0+ns)."""
    psv = ps3[:, s0:s0 + ns, :]
    u1_ps = psv[:, :, 0:N1]                      # [128,ns,8] in PSUM
    r1_ps = psv[:, :, N1:N1 + N2]                # [128,ns,48]
    r2_ps = psv[:, :, N1 + N2:F]                 # [128,ns,192]

    out_sb = out_pool.tile([128, ns * F], f32)
    out3 = out_sb[:].rearrange("p (s f) -> p s f", s=ns)

    e1 = ep_pool.tile([128, ns * N2], f32, tag="e1")
    e1_3 = e1[:].rearrange("p (s w) -> p s w", s=ns)
    e1_4 = e1[:].rearrange("p (s g c) -> p s g c", s=ns, g=C1)
    s1 = ep_pool.tile([128, ns * N1], f32, tag="s1")
    s1_3 = s1[:].rearrange("p (s g) -> p s g", s=ns)
    w1 = ep_pool.tile([128, ns * N1], f32, tag="w1")
    w1_3 = w1[:].rearrange("p (s g) -> p s g", s=ns)
    tmp2 = ep_pool.tile([128, ns * N3], f32, tag="tmp2")
    tmp2_3 = tmp2[:].rearrange("p (s w) -> p s w", s=ns)
    tmp2_4 = tmp2[:].rearrange("p (s g c) -> p s g c", s=ns, g=N2)
    l2 = ep_pool.tile([128, ns * N3], f32, tag="l2")
    l2_3 = l2[:].rearrange("p (s w) -> p s w", s=ns)
    e2 = ep_pool.tile([128, ns * N3], f32, tag="e2")
    e2_4 = e2[:].rearrange("p (s g c) -> p s g c", s=ns, g=N2)
    s2 = ep_pool.tile([128, ns * N2], f32, tag="s2")
    s2_3 = s2[:].rearrange("p (s g) -> p s g", s=ns)
    w2 = ep_pool.tile([128, ns * N2], f32, tag="w2")
    w2_3 = w2[:].rearrange("p (s g) -> p s g", s=ns)

    # u1 -> output cols 0:8
    nc.scalar.copy(out3[:, :, 0:N1], u1_ps)
    # e1 = exp(l1)  (level-1 logits come straight out of PSUM)
    nc.scalar.activation(e1_3, r1_ps, mybir.ActivationFunctionType.Exp)
    # s1[g] = sum_c e1[g,c]
    nc.vector.reduce_sum(s1_3, e1_4, axis=mybir.AxisListType.X)
    # w1 = u1 / s1  (softmax scale * upper), via fast reciprocal
    nc.vector.reciprocal_approx_fast(s1_3, s1_3)
    nc.vector.tensor_mul(w1_3, s1_3, u1_ps)
    # u2 = e1 * bcast6(w1) -> output cols 8:56
    u2_out = out3[:, :, N1:N1 + N2]
    u2_4 = u2_out.rearrange("p s (g c) -> p s g c", g=C1)
    nc.vector.tensor_mul(
        u2_4, e1_4, w1_3[:, :, :, None].broadcast_to([128, ns, C1, C2])
    )
    # l2 = raw2 + bcast4(u2) * wu2
    nc.vector.tensor_mul(
        tmp2_4,
        u2_4.rearrange("p s g c -> p s (g c)")[:, :, :, None]
        .broadcast_to([128, ns, N2, C3]),
        wu2_sb[:].rearrange("p (g c) -> p g c", g=N2)[:, None, :, :]
        .broadcast_to([128, ns, N2, C3]),
    )
    nc.vector.tensor_add(l2_3, r2_ps, tmp2_3)
    # e2 = exp(l2)
    nc.scalar.activation(e2[:], l2[:], mybir.ActivationFunctionType.Exp)
    # s2[g] = sum_c e2[g,c] ; w2 = u2 / s2
    nc.vector.reduce_sum(s2_3, e2_4, axis=mybir.AxisListType.X)
    nc.vector.reciprocal_approx_fast(s2_3, s2_3)
    nc.vector.tensor_mul(w2_3, s2_3, u2_4.rearrange("p s g c -> p s (g c)"))
    # u3 = e2 * bcast4(w2) -> output cols 56:248
    u3_4 = out3[:, :, N1 + N2:F].rearrange("p s (g c) -> p s g c", g=N2)
    nc.vector.tensor_mul(
        u3_4, e2_4, w2_3[:, :, :, None].broadcast_to([128, ns, N2, C3])
    )

    # ---- store [128, ns, 248] -> y rows
    r0 = r_base + s0 * 128
    y_g = y[r0:r0 + ns * 128, :].rearrange("(s p) f -> p s f", p=128)
    nc.sync.dma_start(y_g, out3)


def _get_nc(with_bias):
    key = bool(with_bias)
    if key in _CACHED_NC:
        return _CACHED_NC[key]
    nc = bacc.Bacc("TRN2", target_bir_lowering=False, debug=False,
                   num_devices=NCORES)
    xt = nc.dram_tensor("xt", [D, BC], f32, kind="ExternalInput").ap()
    w = nc.dram_tensor("w", [D, FP], f16, kind="ExternalInput").ap()
    wu2r = nc.dram_tensor("wu2r", [128, N3], f32, kind="ExternalInput").ap()
    if with_bias:
        br = nc.dram_tensor("br", [1, FP], f16, kind="ExternalInput").ap()
        onesr = nc.dram_tensor("onesr", [1, 128], f16, kind="ExternalInput").ap()
    else:
        br = onesr = None
    y = nc.dram_tensor("y", [BC, F], f32, kind="ExternalOutput").ap()
    with tile.TileContext(nc) as tc, ExitStack() as ctx:
        _build_body(ctx, tc, nc, xt, w, wu2r, br, onesr, y, with_bias)
    nc.compile()
    _CACHED_NC[key] = nc
    return nc


def kernel(x, W_final, b_final, Wp1, wu1, b1, Wp2, wu2, b2):
    global LAST_RESULT
    x = np.asarray(x, np.float32)
    W_final = np.asarray(W_final, np.float64)
    b_final = np.asarray(b_final, np.float64)
    Wp1 = np.asarray(Wp1, np.float64)
    wu1 = np.asarray(wu1, np.float64)
    b1 = np.asarray(b1, np.float64)
    Wp2 = np.asarray(Wp2, np.float64)
    wu2 = np.asarray(wu2, np.float64)
    b2 = np.asarray(b2, np.float64)

    # Fold the (linear-in-x) level-1 upper term into the stacked weights.
    Wp1_eff = Wp1 + W_final.T[:, :, None] * wu1[:, None, :]     # (C1, D, C2)
    b1_eff = b1 + b_final[:, None] * wu1                        # (C1, C2)
    W_all = np.concatenate(
        [
            W_final,                                            # (D, 8)
            Wp1_eff.transpose(1, 0, 2).reshape(D, N2),          # (D, 48)
            Wp2.transpose(2, 0, 1, 3).reshape(D, N3),           # (D, 192)
        ],
        axis=1,
    )
    w_pad = np.zeros((D, FP), np.float16)
    w_pad[:, :F] = W_all.astype(np.float16)
    bias_row = np.zeros((1, FP), np.float32)
    bias_row[0, :N1] = b_final
    bias_row[0, N1:N1 + N2] = b1_eff.reshape(-1)
    bias_row[0, N1 + N2:F] = b2.reshape(-1)
    with_bias = bool(np.any(bias_row))
    wu2_rep = np.tile(wu2.reshape(1, N3).astype(np.float32), (128, 1))

    nc = _get_nc(with_bias)
    in_maps = []
    for c in range(NCORES):
        xts = np.ascontiguousarray(x[c * BC:(c + 1) * BC, :].T)
        im = {"xt": xts, "w": w_pad, "wu2r": wu2_rep}
        if with_bias:
            im["br"] = bias_row.astype(np.float16)
            im["onesr"] = np.ones((1, 128), np.float16)
        in_maps.append(im)
    res = bass_utils.run_bass_kernel_spmd(nc, in_maps, core_ids=list(range(NCORES)))
    LAST_RESULT = res
    return np.concatenate([res.results[c]["y"] for c in range(NCORES)], axis=0)


# revision 21
# speedup vs baseline: 1.0865x; 1.0865x over previous
"""Trainium2 Bass kernel for the 3-level soft decision-tree head.

Math (see reference): with pen = x,
  u1 = x @ W_final + b_final                            (B, 8)
  l1 = x @ Wp1 + u1*wu1 + b1 ; p1 = softmax(l1)         (B, 8, 6)
  u2 = p1 * u1
  l2 = x @ Wp2 + u2*wu2 + b2 ; p2 = softmax(l2)         (B, 8, 6, 4)
  u3 = p2 * u2
  out = concat([u1, u2.flat, u3.flat])                  (B, 248)

Key restructurings:
  * The u1*wu1 term is linear in x, so it folds into the level-1 weights
    host-side: Wp1_eff[i,:,c] = Wp1[i,:,c] + W_final[:,i]*wu1[i,c] (and
    b1_eff = b1 + b_final*wu1). All three linears then stack into one
    (1280, 248) matrix -> a single x @ W_all matmul feeds every level.
  * The matmul runs in fp16 (1 cycle/row on the PE + fast weight load;
    fp32/f32r stream at <=half rate). x is loaded from HBM in fp32 and
    cast to fp16 on-chip (ScalarE/GpSimd, which are otherwise idle).
    fp16 keeps 11 mantissa bits; inputs are O(1) and weights O(0.02), so
    there is no range risk and the output error stays ~1e-4 relative.
  * Sharding: pure data parallel, batch 16384 -> 8 x 2048 (one shard per
    NeuronCore). Weights (<2MB) are replicated. The x shard is staged
    host-side in [D, Bc] (transposed) layout so the device can DMA
    [d-on-partition] matmul operand tiles directly; bytes moved are
    identical to the natural layout.
  * Nonzero biases would enter through one extra K=1 matmul (ones row x
    bias row); it is skipped when the biases are all zero (as here).
  * Softmax skips the max-subtraction: logits are O(1) by construction,
    exp cannot overflow, and softmax is shift-invariant so the result is
    unchanged up to fp32 rounding.
"""

import sys

sys.path.insert(0, "/opt/trn_rl_repo")

from contextlib import ExitStack

import numpy as np

import concourse.bass as bass  # noqa: F401
import concourse.mybir as mybir
import concourse.tile as tile
from concourse import bacc, bass_utils

B, D = 16384, 1280
C1, C2, C3 = 8, 6, 4
N1 = C1                 # 8    level-1 (root) outputs
N2 = C1 * C2            # 48   level-2 logits
N3 = C1 * C2 * C3       # 192  level-3 logits
F = N1 + N2 + N3        # 248  output columns
FP = 256                # matmul free dim padded to a friendly size
NCORES = 8
BC = B // NCORES        # 2048 batch rows per core
NGROUPS = 4
R = BC // NGROUPS       # 512 rows per pipeline group
S = R // 128            # 4 subtiles of 128 rows
KC = D // 128           # 10 contraction chunks

f32 = mybir.dt.float32
f16 = mybir.dt.float16

LAST_RESULT = None      # BassKernelResults of the most recent run (for test.py)
GROUPS = None           # set while building: (row0, nrows, pieces)
_CACHED_NC = {}


def _build_body(ctx, tc, nc, xt, w, wu2r, br, onesr, y, with_bias):
    const_pool = ctx.enter_context(tc.tile_pool(name="const", bufs=1))
    xf_pool = ctx.enter_context(tc.tile_pool(name="xf", bufs=1))
    xh_pool = ctx.enter_context(tc.tile_pool(name="xh", bufs=1))
    psum_pool = ctx.enter_context(tc.tile_pool(name="ps", bufs=3, space="PSUM"))
    warm_pool = ctx.enter_context(tc.tile_pool(name="warm", bufs=1, space="PSUM"))
    ep_pool = ctx.enter_context(tc.tile_pool(name="ep", bufs=2))
    out_pool = ctx.enter_context(tc.tile_pool(name="out", bufs=2))

    # Resident constants (issued on the ACT HWDGE ring so the SyncE ring is
    # free to start streaming x immediately)
    w_sb = const_pool.tile([128, KC * FP], f16)
    w_sb3 = w_sb[:].rearrange("p (c n) -> p c n", c=KC)
    nc.scalar.dma_start(w_sb3, w.rearrange("(c p) n -> p c n", p=128))
    wu2_sb = const_pool.tile([128, N3], f32)
    nc.scalar.dma_start(wu2_sb[:], wu2r)
    if with_bias:
        br_sb = const_pool.tile([1, FP], f16)
        nc.scalar.dma_start(br_sb[:], br)
        ones_sb = const_pool.tile([1, 128], f16)
        nc.scalar.dma_start(ones_sb[:], onesr)

    xt_v = xt.rearrange("(c p) b -> p c b", p=128)

    # ---- group schedule: (row0, nrows, load pieces as chunk ranges).
    # Group 0 starts with small pieces so the PE gets going early; the LAST
    # group is small with a tiny final piece, because everything loaded last
    # sits on the kernel's critical tail (cast -> matmul -> softmax -> store
    # all serialize after the final byte arrives).
    global GROUPS
    GROUPS = [
        (0,    512, [(0, 2), (2, 5), (5, 8), (8, 10)]),
        (512,  512, [(0, 5), (5, 10)]),
        (1024, 512, [(0, 5), (5, 10)]),
        (1536, 256, [(0, 5), (5, 10)]),
        (1792, 256, [(0, 4), (4, 8), (8, 10)]),
    ]

    # Everything is prefetched (whole shard fits in SBUF) so the PE never
    # starves once running. Casts alternate DVE/ACT; the last group's casts
    # stay on DVE (cheaper per op -> shorter tail).
    xh_views = {}           # g -> list of (chunk_lo, 3d fp16 view)
    cast_flip = 0
    for g, (r0, nr, pieces) in enumerate(GROUPS):
        views = []
        for (c0, c1) in pieces:
            nch = c1 - c0
            xf = xf_pool.tile([128, nch * nr], f32, tag=f"xf{g}_{c0}")
            xf3 = xf[:].rearrange("p (c b) -> p c b", c=nch)
            nc.sync.dma_start(xf3, xt_v[:, c0:c1, r0:r0 + nr])
            xh = xh_pool.tile([128, nch * nr], f16, tag=f"xh{g}_{c0}")
            if g == len(GROUPS) - 1 or cast_flip % 2 == 0:
                nc.vector.tensor_copy(xh[:], xf[:])
            else:
                nc.scalar.copy(xh[:], xf[:])
            cast_flip += 1
            views.append((c0, xh[:].rearrange("p (c b) -> p c b", c=nch)))
        xh_views[g] = views

    def lhsT(g, k, s):
        for c0, v in xh_views[g]:
            if c0 <= k < c0 + v.shape[1]:
                return v[:, k - c0, s * 128:(s + 1) * 128]
        raise AssertionError

    # ---- PE warm-up: ~45 throwaway matmuls on the (early-resident) weight
    # tile keep TensorE busy >3.4us while x streams in, so the HAM clock
    # gate opens (1.2 -> 2.4 GHz) before the first real matmul. PE is
    # otherwise idle during this window; micro-gaps later never reach the
    # ~3.4us re-throttle window, so the array stays warm for the whole run.
    warm_ps = warm_pool.tile([128, FP], f32)
    for _ in range(45):
        nc.tensor.matmul(warm_ps[:], w_sb3[:, 0, 0:128], w_sb3[:, 0, :],
                         start=True, stop=True)

    for g, (r0, nr, pieces) in enumerate(GROUPS):
        ns_g = nr // 128
        ps = psum_pool.tile([128, ns_g * FP], f32, tag="ps")
        ps3 = ps[:].rearrange("p (s n) -> p s n", s=ns_g)
        for s in range(ns_g):
            for k in range(KC):
                nc.tensor.matmul(
                    ps3[:, s, :],
                    lhsT(g, k, s),
                    w_sb3[:, k, :],
                    start=(k == 0),
                    stop=(not with_bias) and (k == KC - 1),
                )
            if with_bias:
                nc.tensor.matmul(
                    ps3[:, s, :],
                    ones_sb[0:1, :],
                    br_sb[0:1, :],
                    start=False,
                    stop=True,
                )

        # ---- epilogue + store (bank-pair halves for the bigger groups so
        # DVE work granularity stays moderate)
        if ns_g > 4:
            h = ns_g // 2
            _epilogue(nc, ep_pool, out_pool, wu2_sb, ps3, y, r0, 0, h)
            _epilogue(nc, ep_pool, out_pool, wu2_sb, ps3, y, r0, h, ns_g - h)
        else:
            _epilogue(nc, ep_pool, out_pool, wu2_sb, ps3, y, r0, 0, ns_g)


def _epilogue(nc, ep_pool, out_pool, wu2_sb, ps3, y, r_base, s0, ns):
    """Softmax levels 1+2 and output assembly for subtiles [s0, s0+ns)."""
    psv = ps3[:, s0:s0 + ns, :]
    u1_ps = psv[:, :, 0:N1]                      # [128,ns,8] in PSUM
    r1_ps = psv[:, :, N1:N1 + N2]                # [128,ns,48]
    r2_ps = psv[:, :, N1 + N2:F]                 # [128,ns,192]

    out_sb = out_pool.tile([128, ns * F], f32)
    out3 = out_sb[:].rearrange("p (s f) -> p s f", s=ns)

    e1 = ep_pool.tile([128, ns * N2], f32, tag="e1")
    e1_3 = e1[:].rearrange("p (s w) -> p s w", s=ns)
    e1_4 = e1[:].rearrange("p (s g c) -> p s g c", s=ns, g=C1)
    s1 = ep_pool.tile([128, ns * N1], f32, tag="s1")
    s1_3 = s1[:].rearrange("p (s g) -> p s g", s=ns)
    w1 = ep_pool.tile([128, ns * N1], f32, tag="w1")
    w1_3 = w1[:].rearrange("p (s g) -> p s g", s=ns)
    tmp2 = ep_pool.tile([128, ns * N3], f32, tag="tmp2")
    tmp2_3 = tmp2[:].rearrange("p (s w) -> p s w", s=ns)
    tmp2_4 = tmp2[:].rearrange("p (s g c) -> p s g c", s=ns, g=N2)
    l2 = ep_pool.tile([128, ns * N3], f32, tag="l2")
    l2_3 = l2[:].rearrange("p (s w) -> p s w", s=ns)
    e2 = ep_pool.tile([128, ns * N3], f32, tag="e2")
    e2_4 = e2[:].rearrange("p (s g c) -> p s g c", s=ns, g=N2)
    s2 = ep_pool.tile([128, ns * N2], f32, tag="s2")
    s2_3 = s2[:].rearrange("p (s g) -> p s g", s=ns)
    w2 = ep_pool.tile([128, ns * N2], f32, tag="w2")
    w2_3 = w2[:].rearrange("p (s g) -> p s g", s=ns)

    # u1 -> output cols 0:8
    nc.scalar.copy(out3[:, :, 0:N1], u1_ps)
    # e1 = exp(l1)  (level-1 logits come straight out of PSUM)
    nc.scalar.activation(e1_3, r1_ps, mybir.ActivationFunctionType.Exp)
    # s1[g] = sum_c e1[g,c]
    nc.vector.reduce_sum(s1_3, e1_4, axis=mybir.AxisListType.X)
    # w1 = u1 / s1  (softmax scale * upper), via fast reciprocal
    nc.vector.reciprocal_approx_fast(s1_3, s1_3)
    nc.vector.tensor_mul(w1_3, s1_3, u1_ps)
    # u2 = e1 * bcast6(w1) -> output cols 8:56
    u2_out = out3[:, :, N1:N1 + N2]
    u2_4 = u2_out.rearrange("p s (g c) -> p s g c", g=C1)
    nc.vector.tensor_mul(
        u2_4, e1_4, w1_3[:, :, :, None].broadcast_to([128, ns, C1, C2])
    )
    # l2 = raw2 + bcast4(u2) * wu2
    nc.vector.tensor_mul(
        tmp2_4,
        u2_4.rearrange("p s g c -> p s (g c)")[:, :, :, None]
        .broadcast_to([128, ns, N2, C3]),
        wu2_sb[:].rearrange("p (g c) -> p g c", g=N2)[:, None, :, :]
        .broadcast_to([128, ns, N2, C3]),
    )
    nc.vector.tensor_add(l2_3, r2_ps, tmp2_3)
    # e2 = exp(l2)
    nc.scalar.activation(e2[:], l2[:], mybir.ActivationFunctionType.Exp)
    # s2[g] = sum_c e2[g,c] ; w2 = u2 / s2
    nc.vector.reduce_sum(s2_3, e2_4, axis=mybir.AxisListType.X)
    nc.vector.reciprocal_approx_fast(s2_3, s2_3)
    nc.vector.tensor_mul(w2_3, s2_3, u2_4.rearrange("p s g c -> p s (g c)"))
    # u3 = e2 * bcast4(w2) -> output cols 56:248
    u3_4 = out3[:, :, N1 + N2:F].rearrange("p s (g c) -> p s g c", g=N2)
    nc.vector.tensor_mul(
        u3_4, e2_4, w2_3[:, :, :, None].broadcast_to([128, ns, N2, C3])
    )

    # ---- store [128, ns, 248] -> y rows
    r0 = r_base + s0 * 128
    y_g = y[r0:r0 + ns * 128, :].rearrange("(s p) f -> p s f", p=128)
    nc.sync.dma_start(y_g, out3)


def _get_nc(with_bias):
    key = bool(with_bias)
    if key in _CACHED_NC:
        return _CACHED_NC[key]
    nc = bacc.Bacc("TRN2", target_bir_lowering=False, debug=False,
                   num_devices=NCORES)
    xt = nc.dram_tensor("xt", [D, BC], f32, kind="ExternalInput").ap()
    w = nc.dram_tensor("w", [D, FP], f16, kind="ExternalInput").ap()
    wu2r = nc.dram_tensor("wu2r", [128, N3], f32, kind="ExternalInput").ap()
    if with_bias:
        br = nc.dram_tensor("br", [1, FP], f16, kind="ExternalInput").ap()
        onesr = nc.dram_tensor("onesr", [1, 128], f16, kind="ExternalInput").ap()
    else:
        br = onesr = None
    y = nc.dram_tensor("y", [BC, F], f32, kind="ExternalOutput").ap()
    with tile.TileContext(nc) as tc, ExitStack() as ctx:
        _build_body(ctx, tc, nc, xt, w, wu2r, br, onesr, y, with_bias)
    nc.compile()
    _CACHED_NC[key] = nc
    return nc


def kernel(x, W_final, b_final, Wp1, wu1, b1, Wp2, wu2, b2):
    global LAST_RESULT
    x = np.asarray(x, np.float32)
    W_final = np.asarray(W_final, np.float64)
    b_final = np.asarray(b_final, np.float64)
    Wp1 = np.asarray(Wp1, np.float64)
    wu1 = np.asarray(wu1, np.float64)
    b1 = np.asarray(b1, np.float64)
    Wp2 = np.asarray(Wp2, np.float64)
    wu2 = np.asarray(wu2, np.float64)
    b2 = np.asarray(b2, np.float64)

    # Fold the (linear-in-x) level-1 upper term into the stacked weights.
    Wp1_eff = Wp1 + W_final.T[:, :, None] * wu1[:, None, :]     # (C1, D, C2)
    b1_eff = b1 + b_final[:, None] * wu1                        # (C1, C2)
    W_all = np.concatenate(
        [
            W_final,                                            # (D, 8)
            Wp1_eff.transpose(1, 0, 2).reshape(D, N2),          # (D, 48)
            Wp2.transpose(2, 0, 1, 3).reshape(D, N3),           # (D, 192)
        ],
        axis=1,
    )
    w_pad = np.zeros((D, FP), np.float16)
    w_pad[:, :F] = W_all.astype(np.float16)
    bias_row = np.zeros((1, FP), np.float32)
    bias_row[0, :N1] = b_final
    bias_row[0, N1:N1 + N2] = b1_eff.reshape(-1)
    bias_row[0, N1 + N2:F] = b2.reshape(-1)
    with_bias = bool(np.any(bias_row))
    wu2_rep = np.tile(wu2.reshape(1, N3).astype(np.float32), (128, 1))

    nc = _get_nc(with_bias)
    in_maps = []
    for c in range(NCORES):
        xts = np.ascontiguousarray(x[c * BC:(c + 1) * BC, :].T)
        im = {"xt": xts, "w": w_pad, "wu2r": wu2_rep}
        if with_bias:
            im["br"] = bias_row.astype(np.float16)
            im["onesr"] = np.ones((1, 128), np.float16)
        in_maps.append(im)
    res = bass_utils.run_bass_kernel_spmd(nc, in_maps, core_ids=list(range(NCORES)))
    LAST_RESULT = res
    return np.concatenate([res.results[c]["y"] for c in range(NCORES)], axis=0)


# revision 33
# speedup vs baseline: 1.1657x; 1.0729x over previous
"""Trainium2 Bass kernel for the 3-level soft decision-tree head.

Math (see reference): with pen = x,
  u1 = x @ W_final + b_final                            (B, 8)
  l1 = x @ Wp1 + u1*wu1 + b1 ; p1 = softmax(l1)         (B, 8, 6)
  u2 = p1 * u1
  l2 = x @ Wp2 + u2*wu2 + b2 ; p2 = softmax(l2)         (B, 8, 6, 4)
  u3 = p2 * u2
  out = concat([u1, u2.flat, u3.flat])                  (B, 248)

Key restructurings:
  * The u1*wu1 term is linear in x, so it folds into the level-1 weights
    host-side: Wp1_eff[i,:,c] = Wp1[i,:,c] + W_final[:,i]*wu1[i,c] (and
    b1_eff = b1 + b_final*wu1). All three linears then stack into one
    (1280, 248) matrix -> a single x @ W_all matmul feeds every level.
  * The matmul runs in fp16 (1 cycle/row on the PE + fast weight load;
    fp32/f32r stream at <=half rate). x is loaded from HBM in fp32 and
    cast to fp16 on-chip (ScalarE/GpSimd, which are otherwise idle).
    fp16 keeps 11 mantissa bits; inputs are O(1) and weights O(0.02), so
    there is no range risk and the output error stays ~1e-4 relative.
  * Sharding: pure data parallel, batch 16384 -> 8 x 2048 (one shard per
    NeuronCore). Weights (<2MB) are replicated. The x shard is staged
    host-side in [D, Bc] (transposed) layout so the device can DMA
    [d-on-partition] matmul operand tiles directly; bytes moved are
    identical to the natural layout.
  * Nonzero biases would enter through one extra K=1 matmul (ones row x
    bias row); it is skipped when the biases are all zero (as here).
  * Softmax skips the max-subtraction: logits are O(1) by construction,
    exp cannot overflow, and softmax is shift-invariant so the result is
    unchanged up to fp32 rounding.
"""

import os
import sys
import time

sys.path.insert(0, "/opt/trn_rl_repo")
# Recover automatically if a previous run left a core wedged.
os.environ.setdefault("NEURON_RT_RESET_CORES", "1")

from contextlib import ExitStack

import numpy as np

import concourse.bass as bass  # noqa: F401
import concourse.mybir as mybir
import concourse.tile as tile
from concourse import bacc, bass_utils

B, D = 16384, 1280
C1, C2, C3 = 8, 6, 4
N1 = C1                 # 8    level-1 (root) outputs
N2 = C1 * C2            # 48   level-2 logits
N3 = C1 * C2 * C3       # 192  level-3 logits
F = N1 + N2 + N3        # 248  output columns
FP = 256                # matmul free dim padded to a friendly size
NCORES = 8
BC = B // NCORES        # 2048 batch rows per core
NGROUPS = 4
R = BC // NGROUPS       # 512 rows per pipeline group
S = R // 128            # 4 subtiles of 128 rows
KC = D // 128           # 10 contraction chunks

f32 = mybir.dt.float32
f16 = mybir.dt.float16

LAST_RESULT = None      # BassKernelResults of the most recent run (for test.py)
GROUPS = None           # set while building: (row0, nrows, pieces)
_CACHED_NC = {}


def _build_body(ctx, tc, nc, xt, w, wu2r, br, onesr, y, with_bias):
    const_pool = ctx.enter_context(tc.tile_pool(name="const", bufs=1))
    xf_pool = ctx.enter_context(tc.tile_pool(name="xf", bufs=1))
    xh_pool = ctx.enter_context(tc.tile_pool(name="xh", bufs=1))
    psum_pool = ctx.enter_context(tc.tile_pool(name="ps", bufs=3, space="PSUM"))
    warm_pool = ctx.enter_context(tc.tile_pool(name="warm", bufs=1, space="PSUM"))
    ep_pool = ctx.enter_context(tc.tile_pool(name="ep", bufs=3))
    out_pool = ctx.enter_context(tc.tile_pool(name="out", bufs=3))

    # Resident constants (issued on the ACT HWDGE ring so the SyncE ring is
    # free to start streaming x immediately)
    w_sb = const_pool.tile([128, KC * FP], f16)
    w_sb3 = w_sb[:].rearrange("p (c n) -> p c n", c=KC)
    nc.scalar.dma_start(w_sb3, w.rearrange("(c p) n -> p c n", p=128))
    wu2_sb = const_pool.tile([128, N3], f32)
    nc.scalar.dma_start(wu2_sb[:], wu2r)
    if with_bias:
        br_sb = const_pool.tile([1, FP], f16)
        nc.scalar.dma_start(br_sb[:], br)
        ones_sb = const_pool.tile([1, 128], f16)
        nc.scalar.dma_start(ones_sb[:], onesr)

    xt_v = xt.rearrange("(c p) b -> p c b", p=128)

    # ---- group schedule: (row0, nrows, load pieces as chunk ranges).
    # Group 0 starts with small pieces so the PE gets going early; the LAST
    # group is small with a tiny final piece, because everything loaded last
    # sits on the kernel's critical tail (cast -> matmul -> softmax -> store
    # all serialize after the final byte arrives).
    global GROUPS
    GROUPS = [
        (0,    512, [(0, 2), (2, 5), (5, 8), (8, 10)]),
        (512,  512, [(0, 5), (5, 10)]),
        (1024, 512, [(0, 5), (5, 10)]),
        (1536, 256, [(0, 5), (5, 10)]),
        (1792, 256, [(0, 4), (4, 8), (8, 10)]),
    ]

    # Everything is prefetched (whole shard fits in SBUF) so the PE never
    # starves once running. Casts alternate DVE/ACT; the last group's casts
    # stay on DVE (cheaper per op -> shorter tail).
    xh_views = {}           # g -> list of (chunk_lo, 3d fp16 view)
    cast_flip = 0
    for g, (r0, nr, pieces) in enumerate(GROUPS):
        views = []
        for (c0, c1) in pieces:
            nch = c1 - c0
            xf = xf_pool.tile([128, nch * nr], f32, tag=f"xf{g}_{c0}")
            xf3 = xf[:].rearrange("p (c b) -> p c b", c=nch)
            nc.sync.dma_start(xf3, xt_v[:, c0:c1, r0:r0 + nr])
            xh = xh_pool.tile([128, nch * nr], f16, tag=f"xh{g}_{c0}")
            if g <= 1 or g == len(GROUPS) - 1:
                nc.vector.tensor_copy(xh[:], xf[:])   # DVE idle early; short tail
            else:
                nc.scalar.copy(xh[:], xf[:])          # late casts off DVE
            cast_flip += 1
            views.append((c0, xh[:].rearrange("p (c b) -> p c b", c=nch)))
        xh_views[g] = views

    def lhsT(g, k, s):
        for c0, v in xh_views[g]:
            if c0 <= k < c0 + v.shape[1]:
                return v[:, k - c0, s * 128:(s + 1) * 128]
        raise AssertionError

    # ---- PE warm-up: ~45 throwaway matmuls on the (early-resident) weight
    # tile keep TensorE busy >3.4us while x streams in, so the HAM clock
    # gate opens (1.2 -> 2.4 GHz) before the first real matmul. PE is
    # otherwise idle during this window; micro-gaps later never reach the
    # ~3.4us re-throttle window, so the array stays warm for the whole run.
    warm_ps = warm_pool.tile([128, FP], f32)
    for _ in range(17):
        nc.tensor.matmul(warm_ps[:], w_sb3[:, 0, 0:128], w_sb3[:, 0, :],
                         start=True, stop=True)

    for g, (r0, nr, pieces) in enumerate(GROUPS):
        ns_g = nr // 128
        ps = psum_pool.tile([128, ns_g * FP], f32, tag="ps")
        ps3 = ps[:].rearrange("p (s n) -> p s n", s=ns_g)
        for s in range(ns_g):
            for k in range(KC):
                nc.tensor.matmul(
                    ps3[:, s, :],
                    lhsT(g, k, s),
                    w_sb3[:, k, :],
                    start=(k == 0),
                    stop=(not with_bias) and (k == KC - 1),
                )
            if with_bias:
                nc.tensor.matmul(
                    ps3[:, s, :],
                    ones_sb[0:1, :],
                    br_sb[0:1, :],
                    start=False,
                    stop=True,
                )

        # keep the HAM clock gate open across load-paced gaps: a short run
        # of throwaway matmuls after each group fills PE idle windows so the
        # next group's real matmuls stay at 2.4 GHz
        if g < len(GROUPS) - 1:
            for _ in range(12):
                nc.tensor.matmul(warm_ps[:], w_sb3[:, 0, 0:128],
                                 w_sb3[:, 0, :], start=True, stop=True)

        # ---- epilogue + store (bank-pair halves for the bigger groups so
        # DVE work granularity stays moderate)
        off = g < len(GROUPS) - 1
        if ns_g > 4:
            h = ns_g // 2
            _epilogue(nc, ep_pool, out_pool, wu2_sb, ps3, y, r0, 0, h, off)
            _epilogue(nc, ep_pool, out_pool, wu2_sb, ps3, y, r0, h, ns_g - h, off)
        else:
            _epilogue(nc, ep_pool, out_pool, wu2_sb, ps3, y, r0, 0, ns_g, off)


def _epilogue(nc, ep_pool, out_pool, wu2_sb, ps3, y, r_base, s0, ns, offload=False):
    """Softmax levels 1+2 and output assembly for subtiles [s0, s0+ns)."""
    psv = ps3[:, s0:s0 + ns, :]
    u1_ps = psv[:, :, 0:N1]                      # [128,ns,8] in PSUM
    r1_ps = psv[:, :, N1:N1 + N2]                # [128,ns,48]
    r2_ps = psv[:, :, N1 + N2:F]                 # [128,ns,192]

    out_sb = out_pool.tile([128, ns * F], f32)
    out3 = out_sb[:].rearrange("p (s f) -> p s f", s=ns)

    e1 = ep_pool.tile([128, ns * N2], f32, tag="e1")
    e1_3 = e1[:].rearrange("p (s w) -> p s w", s=ns)
    e1_4 = e1[:].rearrange("p (s g c) -> p s g c", s=ns, g=C1)
    s1 = ep_pool.tile([128, ns * N1], f32, tag="s1")
    s1_3 = s1[:].rearrange("p (s g) -> p s g", s=ns)
    w1 = ep_pool.tile([128, ns * N1], f32, tag="w1")
    w1_3 = w1[:].rearrange("p (s g) -> p s g", s=ns)
    tmp2 = ep_pool.tile([128, ns * N3], f32, tag="tmp2")
    tmp2_3 = tmp2[:].rearrange("p (s w) -> p s w", s=ns)
    tmp2_4 = tmp2[:].rearrange("p (s g c) -> p s g c", s=ns, g=N2)
    l2 = ep_pool.tile([128, ns * N3], f32, tag="l2")
    l2_3 = l2[:].rearrange("p (s w) -> p s w", s=ns)
    e2 = ep_pool.tile([128, ns * N3], f32, tag="e2")
    e2_4 = e2[:].rearrange("p (s g c) -> p s g c", s=ns, g=N2)
    s2 = ep_pool.tile([128, ns * N2], f32, tag="s2")
    s2_3 = s2[:].rearrange("p (s g) -> p s g", s=ns)
    w2 = ep_pool.tile([128, ns * N2], f32, tag="w2")
    w2_3 = w2[:].rearrange("p (s g) -> p s g", s=ns)

    # u1 -> output cols 0:8
    nc.scalar.copy(out3[:, :, 0:N1], u1_ps)
    # e1 = exp(l1)  (level-1 logits come straight out of PSUM)
    nc.scalar.activation(e1_3, r1_ps, mybir.ActivationFunctionType.Exp)
    # s1[g] = sum_c e1[g,c]
    nc.vector.reduce_sum(s1_3, e1_4, axis=mybir.AxisListType.X)
    # w1 = u1 / s1  (softmax scale * upper), via fast reciprocal
    nc.vector.reciprocal_approx_fast(s1_3, s1_3)
    nc.vector.tensor_mul(w1_3, s1_3, u1_ps)
    # u2 = e1 * bcast6(w1) -> output cols 8:56
    u2_out = out3[:, :, N1:N1 + N2]
    u2_4 = u2_out.rearrange("p s (g c) -> p s g c", g=C1)
    nc.vector.tensor_mul(
        u2_4, e1_4, w1_3[:, :, :, None].broadcast_to([128, ns, C1, C2])
    )
    # l2 = raw2 + bcast4(u2) * wu2
    nc.vector.tensor_mul(
        tmp2_4,
        u2_4.rearrange("p s g c -> p s (g c)")[:, :, :, None]
        .broadcast_to([128, ns, N2, C3]),
        wu2_sb[:].rearrange("p (g c) -> p g c", g=N2)[:, None, :, :]
        .broadcast_to([128, ns, N2, C3]),
    )
    nc.vector.tensor_add(l2_3, r2_ps, tmp2_3)
    # e2 = exp(l2)
    nc.scalar.activation(e2[:], l2[:], mybir.ActivationFunctionType.Exp)
    # s2[g] = sum_c e2[g,c] ; w2 = u2 / s2
    nc.vector.reduce_sum(s2_3, e2_4, axis=mybir.AxisListType.X)
    nc.vector.reciprocal_approx_fast(s2_3, s2_3)
    nc.vector.tensor_mul(w2_3, s2_3, u2_4.rearrange("p s g c -> p s (g c)"))
    # u3 = e2 * bcast4(w2) -> output cols 56:248
    u3_4 = out3[:, :, N1 + N2:F].rearrange("p s (g c) -> p s g c", g=N2)
    nc.vector.tensor_mul(
        u3_4, e2_4, w2_3[:, :, :, None].broadcast_to([128, ns, N2, C3])
    )

    # ---- store [128, ns, 248] -> y rows
    r0 = r_base + s0 * 128
    y_g = y[r0:r0 + ns * 128, :].rearrange("(s p) f -> p s f", p=128)
    nc.sync.dma_start(y_g, out3)


def _get_nc(with_bias):
    key = bool(with_bias)
    if key in _CACHED_NC:
        return _CACHED_NC[key]
    nc = bacc.Bacc("TRN2", target_bir_lowering=False, debug=False,
                   num_devices=NCORES)
    xt = nc.dram_tensor("xt", [D, BC], f32, kind="ExternalInput").ap()
    w = nc.dram_tensor("w", [D, FP], f16, kind="ExternalInput").ap()
    wu2r = nc.dram_tensor("wu2r", [128, N3], f32, kind="ExternalInput").ap()
    if with_bias:
        br = nc.dram_tensor("br", [1, FP], f16, kind="ExternalInput").ap()
        onesr = nc.dram_tensor("onesr", [1, 128], f16, kind="ExternalInput").ap()
    else:
        br = onesr = None
    y = nc.dram_tensor("y", [BC, F], f32, kind="ExternalOutput").ap()
    with tile.TileContext(nc) as tc, ExitStack() as ctx:
        _build_body(ctx, tc, nc, xt, w, wu2r, br, onesr, y, with_bias)
    nc.compile()
    _CACHED_NC[key] = nc
    return nc


def kernel(x, W_final, b_final, Wp1, wu1, b1, Wp2, wu2, b2):
    global LAST_RESULT
    x = np.asarray(x, np.float32)
    W_final = np.asarray(W_final, np.float64)
    b_final = np.asarray(b_final, np.float64)
    Wp1 = np.asarray(Wp1, np.float64)
    wu1 = np.asarray(wu1, np.float64)
    b1 = np.asarray(b1, np.float64)
    Wp2 = np.asarray(Wp2, np.float64)
    wu2 = np.asarray(wu2, np.float64)
    b2 = np.asarray(b2, np.float64)

    # Fold the (linear-in-x) level-1 upper term into the stacked weights.
    Wp1_eff = Wp1 + W_final.T[:, :, None] * wu1[:, None, :]     # (C1, D, C2)
    b1_eff = b1 + b_final[:, None] * wu1                        # (C1, C2)
    W_all = np.concatenate(
        [
            W_final,                                            # (D, 8)
            Wp1_eff.transpose(1, 0, 2).reshape(D, N2),          # (D, 48)
            Wp2.transpose(2, 0, 1, 3).reshape(D, N3),           # (D, 192)
        ],
        axis=1,
    )
    w_pad = np.zeros((D, FP), np.float16)
    w_pad[:, :F] = W_all.astype(np.float16)
    bias_row = np.zeros((1, FP), np.float32)
    bias_row[0, :N1] = b_final
    bias_row[0, N1:N1 + N2] = b1_eff.reshape(-1)
    bias_row[0, N1 + N2:F] = b2.reshape(-1)
    with_bias = bool(np.any(bias_row))
    wu2_rep = np.tile(wu2.reshape(1, N3).astype(np.float32), (128, 1))

    nc = _get_nc(with_bias)
    in_maps = []
    for c in range(NCORES):
        xts = np.ascontiguousarray(x[c * BC:(c + 1) * BC, :].T)
        im = {"xt": xts, "w": w_pad, "wu2r": wu2_rep}
        if with_bias:
            im["br"] = bias_row.astype(np.float16)
            im["onesr"] = np.ones((1, 128), np.float16)
        in_maps.append(im)
    try:
        res = bass_utils.run_bass_kernel_spmd(nc, in_maps,
                                              core_ids=list(range(NCORES)))
    except Exception:
        # One retry: a freshly compiled NEFF's first execution occasionally
        # trips an NRT exec fault; the reset-on-load env var clears it.
        time.sleep(2.0)
        res = bass_utils.run_bass_kernel_spmd(nc, in_maps,
                                              core_ids=list(range(NCORES)))
    LAST_RESULT = res
    return np.concatenate([res.results[c]["y"] for c in range(NCORES)], axis=0)
